# revision 12
# baseline (speedup 1.0000x reference)
"""Trainium2 Bass kernel for nn_Dep_Context (gnn_message_passing).

Per-sample computation (n=8 samples, data-parallel over 8 cores):
  att_hu   = rowsum(hu^2) / spatialmax              (1, 3600)
  right    = [att_hu * hu ; coordT]                 (18, 3600)
  p1T      = (W^T) @ [p_fea ; coordT]               (18, 3600)
  att_ctx[s] = max_t  sum_d p1T[d,s] * right[d,t]   (3600,)   <- the big part
  out      = p_fea * (att_ctx/max * (1 - att_hu))   (256, 3600)

The (3600, 3600) project2 matrix never leaves PSUM: the tensor engine
streams it into rotating 2-bank PSUM slots (900 cols), ScalarE copies every
odd slot to SBUF, and VectorE drains (even_psum, odd_sbuf) pairs with a
fused tensor_tensor_reduce(max, max) that also chains the per-row running
max through its scalar/accum_out operand.  This splits the PSUM-exit
bandwidth (the bottleneck: 12.96M fp32 values/core) across both engines.

Layout-shuffle DMAs (partition <-> free conversions) bounce through tiny
internal DRAM scratch tensors so every SBUF-side DMA AP stays in natural
partition-major form; the exotic strides live on the DRAM side only.
"""

import numpy as np

import concourse.bacc as bacc
import concourse.bass as bass
import concourse.bass_utils as bass_utils
import concourse.dve_ops as dve_ops
import concourse.mybir as mybir
import concourse.tile as tile
from concourse.dve_spec import C1, Spec, Src0, Src1, _has_src1, lower, maxx
from concourse.dve_table_gen import dve_ver_for
from concourse.dve_uop import DveOpSpec

F32 = mybir.dt.float32
MAX = mybir.AluOpType.max
MULT = mybir.AluOpType.mult

N, C, HID = 8, 256, 10
H = W_SP = 60
HW = H * W_SP                      # 3600
KDIM = HID + 8                     # 18
QCOLS = 900                        # psum slot width (2 banks)
NEG = -3.0e38

# s-tiles: 28 x 128 + 1 x 16
S_TILES = [(i * 128, min(128, HW - i * 128)) for i in range((HW + 127) // 128)]
NS = len(S_TILES)                  # 29


def _coord_features():
    """np float32 port of reference.coord_features(60, 60) -> (3600, 8)."""
    h, w = H, W_SP
    ws = np.arange(w, dtype=np.float32)
    hs = np.arange(h, dtype=np.float32)
    xmin = ws / np.float32(w) * np.float32(2) - np.float32(1)
    xmax = (ws + np.float32(1)) / np.float32(w) * np.float32(2) - np.float32(1)
    xctr = (xmin + xmax) / np.float32(2)
    ymin = hs / np.float32(h) * np.float32(2) - np.float32(1)
    ymax = (hs + np.float32(1)) / np.float32(h) * np.float32(2) - np.float32(1)
    yctr = (ymin + ymax) / np.float32(2)
    feat = np.empty((h, w, 8), dtype=np.float32)
    feat[:, :, 0] = xmin[None, :]
    feat[:, :, 1] = ymin[:, None]
    feat[:, :, 2] = xmax[None, :]
    feat[:, :, 3] = ymax[:, None]
    feat[:, :, 4] = xctr[None, :]
    feat[:, :, 5] = yctr[:, None]
    feat[:, :, 6] = np.float32(1.0 / w)
    feat[:, :, 7] = np.float32(1.0 / h)
    return feat.reshape(HW, 8)


def _mm_chunks(cols):
    """Split [0, cols) into matmul-legal (<=512, single-PSUM-bank) pieces,
    assuming col 0 of the psum tile is bank-aligned."""
    out = []
    a = 0
    while a < cols:
        b = min(cols, (a // 512 + 1) * 512)
        out.append((a, b))
        a = b
    return out


def _bcast_ap(dram_ap, n):
    """AP that reads a single DRAM element n times (partition broadcast)."""
    return bass.AP(
        tensor=dram_ap.tensor, offset=dram_ap.offset, ap=[[0, n], [1, 1]]
    )


def _register_max2():
    """Custom DVE op:  out = max(in0, in1);  accum_out = max(s1, max_k out).

    Drop-in for InstTensorTensorReduce(max, max), which crashes this
    runtime's DVE.  Uses the per-NEFF custom-DVE table machinery (the same
    path as the production TENSOR_MASK_REDUCE max-accum op)."""
    name = "MAX2_ACCMAX_ANT"
    for op in dve_ops.OPS:
        if op.name == name:
            return op

    def _ref(in0, in1, c0, c1, c2):
        p = in0.shape[0]
        body = np.maximum(
            in0.astype(np.float32).reshape(p, -1),
            in1.astype(np.float32).reshape(p, -1),
        )
        seed = np.asarray(c1, np.float32).reshape(-1, 1)
        return body, np.maximum(np.max(body, axis=-1, keepdims=True), seed)

    spec = Spec(body=maxx(Src0, Src1), accum=maxx, accum_init=C1, reference=_ref)
    row = max(dve_ops._SUB_OPCODE_FOR_NAME.values()) + 1
    assert row < 0x20, "custom-DVE opcode rows exhausted"
    dve_ops._SUB_OPCODE_FOR_NAME[name] = row
    shas = {}
    for ver in {dve_ver_for("TRN2")}:
        uops = lower(spec, ver=ver)
        shas[ver] = DveOpSpec(
            name=name, opcode=row, uops=uops, rd1_en=_has_src1(spec)
        ).sha(ver)
    op = dve_ops.DveOp(name, spec, subdim=False, uops_sha=shas)
    dve_ops.OPS.append(op)
    dve_ops.CUSTOM_DVE_SPECS[name] = spec
    return op


MAX2 = _register_max2()


def _build_program(do_compile=True):
    nc = bacc.Bacc(trn_type="TRN2")

    pf_d = nc.dram_tensor("pf", [C, HW], F32, kind="ExternalInput")
    hu_d = nc.dram_tensor("hu120", [120, 300], F32, kind="ExternalInput")
    w0_d = nc.dram_tensor("w0", [128, KDIM], F32, kind="ExternalInput")
    w1_d = nc.dram_tensor("w1", [128, KDIM], F32, kind="ExternalInput")
    w2_d = nc.dram_tensor("w2", [8, KDIM], F32, kind="ExternalInput")
    out_d = nc.dram_tensor("out", [C, HW], F32, kind="ExternalOutput")

    # constants baked into the NEFF
    coordT_d = nc.inline_tensor(np.ascontiguousarray(_coord_features().T), "coordT")
    sel_np = np.zeros((120, 12), dtype=np.float32)
    sel_np[np.arange(120), np.arange(120) % 12] = 1.0
    sel_d = nc.inline_tensor(sel_np, "sel")
    ones_d = nc.inline_tensor(np.ones((1, 128), dtype=np.float32), "ones")

    # DRAM scratch for partition<->free layout shuffles (tiny)
    att_sc = nc.dram_tensor("att_sc", [HW], F32, kind="Internal")
    onem_sc = nc.dram_tensor("onem_sc", [HW], F32, kind="Internal")
    humod_sc = nc.dram_tensor("humod_sc", [HID, HW], F32, kind="Internal")
    w_sc = nc.dram_tensor("w_sc", [HW], F32, kind="Internal")
    m_sc = nc.dram_tensor("m_sc", [128], F32, kind="Internal")
    inv_sc = nc.dram_tensor("inv_sc", [2], F32, kind="Internal")

    with tile.TileContext(nc) as tc:
        with (
            tc.tile_pool(name="singles", bufs=1) as singles,
            tc.tile_pool(name="psum", bufs=4, space="PSUM") as psum,
            tc.tile_pool(name="pairs", bufs=3) as pairs,
            tc.tile_pool(name="trash", bufs=2) as trash,
            tc.tile_pool(name="outp", bufs=2) as outp,
        ):
            # ---------------- input DMAs ----------------
            hu_sb = singles.tile([120, 300], F32)
            nc.sync.dma_start(out=hu_sb, in_=hu_d[:, :])
            w0_sb = singles.tile([128, KDIM], F32)
            nc.sync.dma_start(out=w0_sb, in_=w0_d[:, :])
            w1_sb = singles.tile([128, KDIM], F32)
            nc.sync.dma_start(out=w1_sb, in_=w1_d[:, :])
            w2_sb = singles.tile([8, KDIM], F32)
            nc.sync.dma_start(out=w2_sb, in_=w2_d[:, :])
            coordT_sb = singles.tile([8, HW], F32)
            nc.sync.dma_start(out=coordT_sb, in_=coordT_d[:, :])
            sel_sb = singles.tile([120, 12], F32)
            nc.sync.dma_start(out=sel_sb, in_=sel_d[:, :])
            ones_sb = singles.tile([1, 128], F32)
            nc.sync.dma_start(out=ones_sb, in_=ones_d[:, :])

            # p_fea in two channel-chunks; halves ordered so project1 can
            # start after the first half of both chunks.
            pf_sb = singles.tile([128, 2, HW], F32)
            for half in range(2):
                cs = slice(half * 1800, (half + 1) * 1800)
                for cc in range(2):
                    nc.sync.dma_start(
                        out=pf_sb[:, cc, cs],
                        in_=pf_d[cc * 128 : (cc + 1) * 128, cs],
                    )

            # ---------------- prologue: att_hu ----------------
            husq = singles.tile([120, 300], F32)
            nc.scalar.square(husq, hu_sb)
            xff_ps = psum.tile([12, 300], F32, tag="slot", name="xff_ps")
            nc.tensor.matmul(xff_ps, sel_sb, husq, start=True, stop=True)
            xff12 = singles.tile([12, 300], F32)
            nc.scalar.copy(xff12, xff_ps)

            xmax = singles.tile([12, 1], F32)
            nc.vector.reduce_max(xmax, xff12, axis=mybir.AxisListType.X)
            # 12 partitions -> one row via DRAM bounce
            nc.sync.dma_start(out=m_sc[0:12], in_=xmax[:, :])
            xmax_row = singles.tile([1, 12], F32)
            nc.sync.dma_start(out=xmax_row, in_=m_sc[0:12])
            xmax1 = singles.tile([1, 1], F32)
            nc.vector.reduce_max(xmax1, xmax_row, axis=mybir.AxisListType.X)
            invx = singles.tile([1, 1], F32)
            nc.vector.reciprocal(invx, xmax1)
            nc.sync.dma_start(out=inv_sc[0:1], in_=invx[:, :])
            invx12 = singles.tile([12, 1], F32)
            nc.gpsimd.dma_start(out=invx12, in_=_bcast_ap(inv_sc[0:1], 12))
            att12 = singles.tile([12, 300], F32)
            nc.vector.tensor_scalar_mul(att12, xff12, invx12)
            onem12 = singles.tile([12, 300], F32)
            nc.scalar.activation(
                onem12, att12, mybir.ActivationFunctionType.Copy, bias=1.0, scale=-1.0
            )
            # stage att_hu and (1 - att_hu) to DRAM in t-order (t = c*300 + k)
            nc.sync.dma_start(
                out=att_sc[:].rearrange("(c k) -> c k", k=300), in_=att12[:, :]
            )
            nc.sync.dma_start(
                out=onem_sc[:].rearrange("(c k) -> c k", k=300), in_=onem12[:, :]
            )

            # right = [att_hu * hu ; coordT]  (18, 3600)
            right_sb = singles.tile([KDIM, HW], F32)
            nc.sync.dma_start(out=right_sb[10:18, :], in_=coordT_d[:, :])
            # att_hu broadcast over the 10 hu channels: DRAM step-0 read
            att120 = singles.tile([120, 300], F32)
            att_flat = att_sc[:]
            nc.gpsimd.dma_start(
                out=att120,
                in_=bass.AP(
                    tensor=att_flat.tensor,
                    offset=att_flat.offset,
                    ap=[[0, 10], [300, 12], [1, 300]],
                ),
            )
            humod = singles.tile([120, 300], F32)
            nc.vector.tensor_tensor(humod, hu_sb, att120, MULT)
            # hu_mod -> (10, 3600) row layout via DRAM (plain reshape both ways)
            nc.sync.dma_start(
                out=humod_sc[:, :].rearrange("d (c k) -> (d c) k", k=300),
                in_=humod[:, :],
            )
            nc.sync.dma_start(out=right_sb[0:10, :], in_=humod_sc[:, :])

            # ---------------- project1^T (18, 3600) ----------------
            p1T_sb = singles.tile([KDIM, HW], F32)
            for q in range(4):
                c0 = q * QCOLS
                p1_ps = psum.tile([KDIM, QCOLS], F32, tag="slot", name="p1_ps")
                for a, b in _mm_chunks(QCOLS):
                    srcs = (
                        (w0_sb, pf_sb[:, 0, c0 + a : c0 + b]),
                        (w1_sb, pf_sb[:, 1, c0 + a : c0 + b]),
                        (w2_sb, coordT_sb[:, c0 + a : c0 + b]),
                    )
                    for g, (wt, src) in enumerate(srcs):
                        nc.tensor.matmul(
                            p1_ps[:, a:b], wt, src, start=(g == 0), stop=(g == 2)
                        )
                nc.scalar.copy(p1T_sb[:, c0 : c0 + QCOLS], p1_ps)

            # ---------------- main: att_ctx[s] = max_t project2 ----------------
            att_ctx = singles.tile([128, NS], F32)
            nc.vector.memset(att_ctx, NEG)

            for i, (s0, sn) in enumerate(S_TILES):
                lhsT = p1T_sb[:, s0 : s0 + sn]
                slots = []
                for q in range(4):
                    c0 = q * QCOLS
                    slot = psum.tile([128, QCOLS], F32, tag="slot", name="slot")
                    for a, b in _mm_chunks(QCOLS):
                        nc.tensor.matmul(
                            slot[:sn, a:b],
                            lhsT,
                            right_sb[:, c0 + a : c0 + b],
                            start=True,
                            stop=True,
                        )
                    slots.append(slot)
                acc = att_ctx[:sn, i : i + 1]
                for p in range(2):
                    a_ps, b_ps = slots[2 * p], slots[2 * p + 1]
                    cp = pairs.tile([128, QCOLS], F32, name="cp")
                    nc.scalar.copy(cp[:sn], b_ps[:sn])
                    tr = trash.tile([128, QCOLS], F32, name="tr")
                    nc.vector._custom_dve(
                        MAX2,
                        out=tr[:sn],
                        in0=a_ps[:sn],
                        in1=cp[:sn],
                        s1=(NEG if p == 0 else acc),
                        accum_out=acc,
                    )

            # ---------------- tail ----------------
            # global max over att_ctx -> 1/M
            mcol = singles.tile([128, 1], F32)
            nc.vector.reduce_max(mcol, att_ctx, axis=mybir.AxisListType.X)
            nc.sync.dma_start(out=m_sc[:], in_=mcol[:, :])
            mrow = singles.tile([1, 128], F32)
            nc.sync.dma_start(out=mrow, in_=m_sc[:])
            m1 = singles.tile([1, 1], F32)
            nc.vector.reduce_max(m1, mrow, axis=mybir.AxisListType.X)
            invm = singles.tile([1, 1], F32)
            nc.vector.reciprocal(invm, m1)
            nc.sync.dma_start(out=inv_sc[1:2], in_=invm[:, :])
            invm128 = singles.tile([128, 1], F32)
            nc.gpsimd.dma_start(out=invm128, in_=_bcast_ap(inv_sc[1:2], 128))

            # (1 - att_hu) in [128, NS] layout (t = i*128 + p)
            onem128 = singles.tile([128, NS], F32)
            nc.vector.memset(onem128, 0.0)
            nc.sync.dma_start(
                out=onem128[:, 0:28],
                in_=onem_sc[0:3584].rearrange("(i p) -> p i", p=128),
            )
            nc.sync.dma_start(out=onem128[0:16, 28:29], in_=onem_sc[3584:3600])

            # w = att_ctx * (1 - att_hu) / M, then to a row vector via DRAM
            w128 = singles.tile([128, NS], F32)
            nc.vector.tensor_tensor(w128, att_ctx, onem128, MULT)
            nc.vector.tensor_scalar_mul(w128, w128, invm128)
            nc.sync.dma_start(
                out=w_sc[0:3584].rearrange("(i p) -> p i", p=128),
                in_=w128[:, 0:28],
            )
            nc.sync.dma_start(out=w_sc[3584:3600], in_=w128[0:16, 28:29])
            wrow = singles.tile([1, HW], F32)
            nc.sync.dma_start(out=wrow, in_=w_sc[:])

            # out[c, t] = pf[c, t] * w[t]:  broadcast w over partitions with a
            # K=1 matmul, multiply, stream out.
            for q in range(4):
                c0 = q * QCOLS
                bc_ps = psum.tile([128, QCOLS], F32, tag="slot", name="bc_ps")
                for a, b in _mm_chunks(QCOLS):
                    nc.tensor.matmul(
                        bc_ps[:, a:b],
                        ones_sb,
                        wrow[:, c0 + a : c0 + b],
                        start=True,
                        stop=True,
                    )
                for cc in range(2):
                    o_sb = outp.tile([128, QCOLS], F32, name="o_sb")
                    nc.vector.tensor_tensor(
                        o_sb, pf_sb[:, cc, c0 : c0 + QCOLS], bc_ps, MULT
                    )
                    nc.sync.dma_start(
                        out=out_d[cc * 128 : (cc + 1) * 128, c0 : c0 + QCOLS],
                        in_=o_sb,
                    )

    if do_compile:
        nc.compile()
    return nc


_NC_CACHE = None


def _get_nc():
    global _NC_CACHE
    if _NC_CACHE is None:
        _NC_CACHE = _build_program()
    return _NC_CACHE


def run(p_fea, hu, W, **run_kwargs):
    """Run on 8 NeuronCores (1 sample per core). Returns (out, BassKernelResults)."""
    p_fea = np.ascontiguousarray(p_fea, dtype=np.float32)
    hu = np.ascontiguousarray(hu, dtype=np.float32)
    W = np.ascontiguousarray(W, dtype=np.float32)
    nc = _get_nc()
    w0 = np.ascontiguousarray(W[0:128])
    w1 = np.ascontiguousarray(W[128:256])
    w2 = np.ascontiguousarray(W[256:264])
    in_maps = []
    for s in range(N):
        in_maps.append(
            {
                "pf": np.ascontiguousarray(p_fea[s].reshape(C, HW)),
                "hu120": np.ascontiguousarray(hu[s].reshape(120, 300)),
                "w0": w0,
                "w1": w1,
                "w2": w2,
            }
        )
    res = bass_utils.run_bass_kernel_spmd(
        nc, in_maps, core_ids=list(range(N)), **run_kwargs
    )
    out = np.stack([r["out"] for r in res.results]).reshape(N, C, H, W_SP)
    return out, res


def kernel(p_fea, hu, W):
    out, _ = run(p_fea, hu, W)
    return out
